# revision 1
# baseline (speedup 1.0000x reference)
"""AttentionMM kernel for Trainium2 (Bass/Tile), data-parallel over 8 NeuronCores.

Math (per batch b, with x1,x2: (T,E)):
    S = x1 @ x2^T  is never materialized:
        t1 = sum_i x1[i,:] ;  t2 = sum_j x2[j,:]
        G2 = x1^T @ x2  (E,E);  G = G2^T
        c1 = (1/T) G2^T t2 ;  c2 = (1/T) G t1   (computed as G2^T t2 / G^T t1)
    et1 = c1 @ U1 + x1 @ W1 + b1 ;  et2 = c2 @ U2 + x2 @ W2 + b2
    o1 = softmax(et1) @ x1 ;  o2 = softmax(et2) @ x2 ;  out = [o1 | o2]

Implementation notes:
  - Tokens sit in SBUF partitions, p-major: token t = p*16 + k, so each
    DMA moves 16 consecutive 516B rows per partition (4KB+ contiguous
    chunks on both sides -> near-peak HBM bandwidth).
  - The host appends a ones-column to x (E -> 129 cols): Gram matmuls
    then yield the token-sums t1/t2 for free, and readout matmuls yield
    the softmax denominator Z for free.
  - Softmax uses a constant shift instead of a max-subtraction (logits
    for this problem are < ~70, so exp stays in fp32 range; a constant
    shift cancels exactly in o = (sum ex*x)/Z).
  - Matmul operands are float32r (single-pass PE matmul). Set
    USE_F32R = False to fall back to full fp32 (two-pass, ~2x PE time).
  - x@W runs on GPSIMD (multiply) + DVE (grouped reduce), keeping the
    PE free; U@c runs batched over all 4 resident batches per core.
"""

import numpy as np

import concourse.bass as bass
import concourse.mybir as mybir
import concourse.tile as tile
from concourse.bass_utils import run_bass_kernel_spmd

B, T, E = 32, 2048, 128
NCORES = 8
BPC = B // NCORES            # batches per core
KT = T // 128                # token tiles per batch
CW = E + 2                   # row width: 128 x-cols + ones col + pad (f32r needs even N)
F32 = mybir.dt.float32
AF = mybir.ActivationFunctionType
ALU = mybir.AluOpType
ET_SHIFT = -40.0             # constant softmax shift (cancels in o)
INV_T = 1.0 / T

USE_F32R = True
MMDT = mybir.dt.float32r if USE_F32R else F32


def _patch_sem_clear():
    """The installed walrus cannot encode EVENT_SEMAPHORE_RANGE_CLEAR (raw
    ISA, "ISA wrong length"), which TileContext's exit path emits via
    gpsimd.sem_clear. Skip the clear (keep the DMA drain + bookkeeping);
    the runtime re-initializes semaphore state per NEFF execution."""
    if getattr(bass.Bass, "_semclear_patched", False):
        return
    from concourse.bass import compact_to_ranges

    def patched(self, sems):
        if not sems:
            return
        sem_nums = [s.num if hasattr(s, "num") else s for s in sems]
        for sem_range in compact_to_ranges(sem_nums):
            assert self._state.free_isdisjoint(sem_range)
            self.gpsimd.dma_reset(sem_range)
        self._state.prepend_free_semaphores(sem_nums)
        for poison_set in self._tile_sem_poison_stack:
            poison_set.update(sem_nums)

    bass.Bass.clear_and_free_semaphores = patched
    bass.Bass._semclear_patched = True


def _legalize_sync_waits(nc):
    """The installed walrus encodes at most one sync-wait per instruction
    ("Too many sync wait commands"). Move excess waits onto engine NoOps
    inserted immediately before the instruction — same engine, same
    program position, so semantics are unchanged."""
    import bass_rust

    fn = nc.m.functions[0]
    n_nops = 0
    for blk in fn.blocks:
        insts = blk.instructions
        out = []
        dirty = False
        for inst in insts:
            si = inst.sync_info
            if si is not None and len(si.on_wait) > 1:
                waits = list(si.on_wait)
                for w in waits[:-1]:
                    nop = mybir.InstNoOp(
                        name=f"waitnop-{n_nops}", engine=inst.engine
                    )
                    nop.sync_info = bass_rust.SyncInfo(
                        on_wait=[w], on_update=[]
                    )
                    out.append(nop)
                    n_nops += 1
                inst.sync_info = bass_rust.SyncInfo(
                    on_wait=[waits[-1]], on_update=list(si.on_update)
                )
                dirty = True
            out.append(inst)
        if dirty:
            blk.instructions = out
    return n_nops


def _build():
    _patch_sem_clear()
    nc = bass.Bass(
        "TRN2", target_bir_lowering=False, debug=False, num_devices=NCORES
    )

    x1d = nc.dram_tensor("x1c", (BPC, T, CW), MMDT, kind="ExternalInput").ap()
    x2d = nc.dram_tensor("x2c", (BPC, T, CW), MMDT, kind="ExternalInput").ap()
    u1d = nc.dram_tensor("u1", (E, T), MMDT, kind="ExternalInput").ap()
    u2d = nc.dram_tensor("u2", (E, T), MMDT, kind="ExternalInput").ap()
    w1d = nc.dram_tensor("w1bc", (128, E), F32, kind="ExternalInput").ap()
    w2d = nc.dram_tensor("w2bc", (128, E), F32, kind="ExternalInput").ap()
    b1d = nc.dram_tensor("b1s", (128, KT), F32, kind="ExternalInput").ap()
    b2d = nc.dram_tensor("b2s", (128, KT), F32, kind="ExternalInput").ap()
    outd = nc.dram_tensor("out", (BPC, 2 * E), F32, kind="ExternalOutput").ap()

    with tile.TileContext(nc) as tc:
        with (
            tc.tile_pool(name="const", bufs=1) as cpool,
            tc.tile_pool(name="xpool", bufs=1) as xpool,
            tc.tile_pool(name="work", bufs=2) as wpool,
            tc.tile_pool(name="ps", bufs=1, space="PSUM") as pspool,
        ):
            # ---- persistent tiles ----
            U1s = cpool.tile([128, T], MMDT, tag="u1")
            U2s = cpool.tile([128, T], MMDT, tag="u2")
            W1bc = cpool.tile([128, E], F32, tag="w1")
            W2bc = cpool.tile([128, E], F32, tag="w2")
            b1s = cpool.tile([128, KT], F32, tag="b1")
            b2s = cpool.tile([128, KT], F32, tag="b2")
            C1all = cpool.tile([128, BPC], MMDT, tag="c1all")
            C2all = cpool.tile([128, BPC], MMDT, tag="c2all")
            OUT = cpool.tile([1, BPC * 2 * E], F32, tag="outbuf")
            shift = cpool.tile([128, 1], F32, tag="shift")
            nc.gpsimd.memset(shift[:], ET_SHIFT)

            # small/param DMAs first on the ACT HWDGE ring
            nc.scalar.dma_start(W1bc[:], w1d)
            nc.scalar.dma_start(W2bc[:], w2d)
            nc.scalar.dma_start(b1s[:], b1d)
            nc.scalar.dma_start(b2s[:], b2d)

            # ---- x tiles: (128, KT, 129), token t = p*16 + k (p-major) ----
            # x1 on the sync ring, x2 on the scalar ring; halves so the
            # Gram loop can start after half a batch has landed.
            X1 = []
            X2 = []
            KH = KT // 2
            for b in range(BPC):
                x1t = xpool.tile([128, KT, CW], MMDT, tag=f"x1_{b}")
                x2t = xpool.tile([128, KT, CW], MMDT, tag=f"x2_{b}")
                x1s = x1d[b].rearrange("(p k) c -> p k c", k=KT)
                x2s = x2d[b].rearrange("(p k) c -> p k c", k=KT)
                # batch 0 in quarters (earlier compute start), rest in halves
                nh = 4 if b == 0 else 2
                kq = KT // nh
                for h in range(nh):
                    ks = slice(h * kq, (h + 1) * kq)
                    nc.sync.dma_start(x1t[:, ks, :], x1s[:, ks])
                    nc.scalar.dma_start(x2t[:, ks, :], x2s[:, ks])
                X1.append(x1t)
                X2.append(x2t)

            # U's load behind the x's (first needed only at the U-phase)
            nc.sync.dma_start(U1s[:], u1d)
            nc.scalar.dma_start(U2s[:], u2d)

            psE = pspool.tile([128, 2 * BPC * KT], F32, tag="psE", bufs=1)

            # ---- per-batch: Gram phases, c's, x@W ----
            for b in range(BPC):
                x1t, x2t = X1[b], X2[b]

                # phase A: [G2 | t1] = x1^T @ [x2 | 1]
                psA = pspool.tile([128, CW], F32, tag="psA", bufs=2)
                for k in range(KT):
                    nc.tensor.matmul(
                        psA[:],
                        x1t[:, k, 0:E],
                        x2t[:, k, :],
                        start=(k == 0),
                        stop=(k == KT - 1),
                    )
                # phase B: [G | t2] = x2^T @ [x1 | 1]
                psB = pspool.tile([128, CW], F32, tag="psB", bufs=2)
                for k in range(KT):
                    nc.tensor.matmul(
                        psB[:],
                        x2t[:, k, 0:E],
                        x1t[:, k, :],
                        start=(k == 0),
                        stop=(k == KT - 1),
                    )

                GA = wpool.tile([128, CW], MMDT, tag="ga", bufs=2)
                GB = wpool.tile([128, CW], MMDT, tag="gb", bufs=2)
                nc.vector.tensor_copy(GA[:], psA[:])
                nc.vector.tensor_copy(GB[:], psB[:])

                # TC = [t1 | t2]; then (f32r needs even N):
                #   lhsT=G2, rhs=TC -> [. | G2^T t2] = [. | T*c1]
                #   lhsT=G,  rhs=TC -> [G^T t1 | .] = [T*c2 | .]
                TC = wpool.tile([128, 2], MMDT, tag="tc", bufs=2)
                nc.vector.tensor_copy(TC[:, 0:1], GA[:, E : E + 1])
                nc.vector.tensor_copy(TC[:, 1:2], GB[:, E : E + 1])
                psC = pspool.tile([128, 4], F32, tag="psC", bufs=1)
                nc.tensor.matmul(psC[:, 0:2], GA[:, 0:E], TC[:], start=True, stop=True)
                nc.tensor.matmul(psC[:, 2:4], GB[:, 0:E], TC[:], start=True, stop=True)
                # scale by 1/T while copying into the batched c matrices
                nc.vector.tensor_scalar_mul(C1all[:, b : b + 1], psC[:, 1:2], INV_T)
                nc.vector.tensor_scalar_mul(C2all[:, b : b + 1], psC[:, 2:3], INV_T)

                # x@W (+b): big multiply on GPSIMD, grouped reduce on DVE
                xwb1 = wpool.tile([128, KT], F32, tag="xwb1", bufs=2)
                xw2 = wpool.tile([128, KT], F32, tag="xw2", bufs=2)
                scr1 = wpool.tile([128, KT, E], F32, tag="scr1", bufs=2)
                scr2 = wpool.tile([128, KT, E], F32, tag="scr2", bufs=2)
                nc.gpsimd.tensor_tensor(
                    scr1[:],
                    x1t[:, :, 0:E].bitcast(F32),
                    W1bc.unsqueeze(1).broadcast_to((128, KT, E)),
                    ALU.mult,
                )
                nc.vector.tensor_reduce(
                    out=xwb1[:], in_=scr1[:], axis=mybir.AxisListType.X, op=ALU.add
                )
                nc.gpsimd.tensor_add(xwb1[:], xwb1[:], b1s[:])
                nc.gpsimd.tensor_tensor(
                    scr2[:],
                    x2t[:, :, 0:E].bitcast(F32),
                    W2bc.unsqueeze(1).broadcast_to((128, KT, E)),
                    ALU.mult,
                )
                nc.vector.tensor_reduce(
                    out=xw2[:], in_=scr2[:], axis=mybir.AxisListType.X, op=ALU.add
                )
                nc.gpsimd.tensor_add(xw2[:], xw2[:], b2s[:])
                X1[b] = (x1t, xwb1)
                X2[b] = (x2t, xw2)

            # ---- U phase: et contributions for all batches at once ----
            # token t = p*16+k  ->  U column for (p, k) is U[:, p*16+k];
            # the host pre-permutes U so tile k's columns are contiguous.
            for k in range(KT):
                nc.tensor.matmul(
                    psE[:, k * BPC : (k + 1) * BPC],
                    U1s[:, k * 128 : (k + 1) * 128],
                    C1all[:],
                    start=True,
                    stop=True,
                )
            for k in range(KT):
                off = BPC * KT
                nc.tensor.matmul(
                    psE[:, off + k * BPC : off + (k + 1) * BPC],
                    U2s[:, k * 128 : (k + 1) * 128],
                    C2all[:],
                    start=True,
                    stop=True,
                )

            psE1 = psE[:, 0 : BPC * KT].rearrange("p (k c) -> p k c", c=BPC)
            psE2 = psE[:, BPC * KT : 2 * BPC * KT].rearrange(
                "p (k c) -> p k c", c=BPC
            )

            # ---- per-batch: logits, exp, readout, normalize ----
            for b in range(BPC):
                x1t, xwb1 = X1[b]
                x2t, xw2 = X2[b]

                et1 = wpool.tile([128, KT], F32, tag="et1", bufs=2)
                et2 = wpool.tile([128, KT], F32, tag="et2", bufs=2)
                nc.vector.scalar_tensor_tensor(
                    out=et1[:],
                    in0=psE1[:, :, b],
                    scalar=1.0,
                    in1=xwb1[:],
                    op0=ALU.mult,
                    op1=ALU.add,
                )
                nc.vector.scalar_tensor_tensor(
                    out=et2[:],
                    in0=psE2[:, :, b],
                    scalar=1.0,
                    in1=xw2[:],
                    op0=ALU.mult,
                    op1=ALU.add,
                )
                EX1 = wpool.tile([128, KT], MMDT, tag="ex1", bufs=2)
                EX2 = wpool.tile([128, KT], MMDT, tag="ex2", bufs=2)
                nc.scalar.activation(EX1[:], et1[:], AF.Exp, bias=shift[:])
                nc.scalar.activation(EX2[:], et2[:], AF.Exp, bias=shift[:])

                # readout: [o~ | Z] = EX^T @ [x | 1], accumulated over k tiles
                psO = pspool.tile([1, 2 * CW], F32, tag="psO", bufs=2)
                for k in range(KT):
                    nc.tensor.matmul(
                        psO[:, 0:CW],
                        EX1[:, k : k + 1],
                        x1t[:, k, :],
                        start=(k == 0),
                        stop=(k == KT - 1),
                    )
                for k in range(KT):
                    nc.tensor.matmul(
                        psO[:, CW : 2 * CW],
                        EX2[:, k : k + 1],
                        x2t[:, k, :],
                        start=(k == 0),
                        stop=(k == KT - 1),
                    )

                # normalize: out = o~ / Z
                rz = wpool.tile([1, 2], F32, tag="rz", bufs=2)
                nc.vector.reciprocal(
                    rz[:], psO[:, :].rearrange("p (s c) -> p s c", c=CW)[:, :, E]
                )
                obase = b * 2 * E
                nc.scalar.mul(OUT[:, obase : obase + E], psO[:, 0:E], rz[:, 0:1])
                nc.scalar.mul(
                    OUT[:, obase + E : obase + 2 * E],
                    psO[:, CW : CW + E],
                    rz[:, 1:2],
                )
                # per-batch store so only the last batch's sits in the tail
                nc.sync.dma_start(
                    outd[b].unsqueeze(0), OUT[:, obase : obase + 2 * E]
                )

    return nc


_NC_CACHE = {}


def _get_nc():
    if "nc" not in _NC_CACHE:
        _NC_CACHE["nc"] = _build()
    return _NC_CACHE["nc"]


# U column permutation: tile k, lane j  <-  U[:, j*16 + k]
_UIDX = np.arange(T).reshape(128, KT).T.reshape(-1)


def _prep_in_maps(x1, x2, W1, b1, U1, W2, b2, U2):
    x1 = np.asarray(x1, dtype=np.float32)
    x2 = np.asarray(x2, dtype=np.float32)
    W1 = np.asarray(W1, dtype=np.float32)
    W2 = np.asarray(W2, dtype=np.float32)
    b1 = np.asarray(b1, dtype=np.float32)
    b2 = np.asarray(b2, dtype=np.float32)
    U1 = np.asarray(U1, dtype=np.float32)
    U2 = np.asarray(U2, dtype=np.float32)

    # append ones column + zero pad: (B, T, E+2)
    pad = np.zeros((B, T, 2), dtype=np.float32)
    pad[:, :, 0] = 1.0
    x1h = np.ascontiguousarray(np.concatenate([x1, pad], axis=2))
    x2h = np.ascontiguousarray(np.concatenate([x2, pad], axis=2))

    w1bc = np.ascontiguousarray(np.broadcast_to(W1[:, 0][None, :], (128, E)))
    w2bc = np.ascontiguousarray(np.broadcast_to(W2[:, 0][None, :], (128, E)))
    # token t = p*16 + k  ->  b1s[p, k]
    b1s = np.ascontiguousarray(b1[:, 0].reshape(128, KT))
    b2s = np.ascontiguousarray(b2[:, 0].reshape(128, KT))
    u1p = np.ascontiguousarray(U1[:, _UIDX])
    u2p = np.ascontiguousarray(U2[:, _UIDX])

    in_maps = []
    for c in range(NCORES):
        sl = slice(c * BPC, (c + 1) * BPC)
        in_maps.append(
            {
                "x1c": np.ascontiguousarray(x1h[sl]),
                "x2c": np.ascontiguousarray(x2h[sl]),
                "u1": u1p,
                "u2": u2p,
                "w1bc": w1bc,
                "w2bc": w2bc,
                "b1s": b1s,
                "b2s": b2s,
            }
        )
    return in_maps


def _run(trace=False, tmpdir=None, **inputs):
    nc = _get_nc()
    if not _NC_CACHE.get("legalized"):
        # must happen after any CoreSim use (sim can't model bare wait-nops)
        _legalize_sync_waits(nc)
        _NC_CACHE["legalized"] = True
    in_maps = _prep_in_maps(**inputs)
    res = run_bass_kernel_spmd(
        nc, in_maps, list(range(NCORES)), trace=trace, tmpdir=tmpdir
    )
    out = np.concatenate([r["out"] for r in res.results], axis=0)
    return out, res


def kernel(x1, x2, W1, b1, U1, W2, b2, U2):
    out, _ = _run(
        x1=x1, x2=x2, W1=W1, b1=b1, U1=U1, W2=W2, b2=b2, U2=U2
    )
    return out



# revision 12
# speedup vs baseline: 1.5338x; 1.5338x over previous
"""AttentionMM kernel for Trainium2 (Bass/Tile), data-parallel over 8 NeuronCores.

Math (per batch b, with x1,x2: (T,E)):
    S = x1 @ x2^T  is never materialized:
        t1 = sum_i x1[i,:] ;  t2 = sum_j x2[j,:]
        G2 = x1^T @ x2  (E,E)
        c1 = (1/T) G2^T t2 ;  c2 = (1/T) G2 t1   (via G = x2^T x1 = G2^T)
    et1 = c1 @ U1 + x1 @ W1 + b1 ;  et2 = c2 @ U2 + x2 @ W2 + b2
    o1 = softmax(et1) @ x1 ;  o2 = softmax(et2) @ x2 ;  out = [o1 | o2]

Implementation notes:
  - Everything on-chip is float16: fp16 matmuls run at 1 cycle/row on the PE
    (vs 4 cycles/row for f32r below 256 moving columns) and halve HBM
    traffic.  PSUM accumulation stays fp32, so the only precision loss is
    input rounding (2^-11) - measured end-to-end rel err ~7e-3.
  - Tokens sit in SBUF partitions, p-major: token t = p*16 + k, so each
    DMA moves 16 consecutive 260B rows per partition (4KB+ contiguous
    chunks on both sides -> good HBM descriptor efficiency).
  - The host appends a ones-column to x (E -> 129 cols, padded to 130):
    Gram matmuls then yield the token-sums t1/t2 for free, and readout
    matmuls yield the softmax denominator Z for free.
  - Softmax uses a constant shift (-27) instead of a max-subtraction.
    The shift cancels exactly in o = (sum ex*x)/Z; -27 keeps every
    batch's exp() inside fp16 range (global max logit ~37 -> e^10~22K
    < 65504; weakest batch max logit ~15 -> e^-11.6 ~ 9e-6, still ~150
    denormal steps).
  - Both attention sides are packed side by side ([x1|1|0|x2|1|0] per
    token) so per-batch elementwise work is one DVE op per stage and the
    readout runs as a single M=2, N=260 accumulating matmul per k-tile.
  - x@W runs on DVE (fused multiply then X-axis reduce, fp16 2x/4x
    modes); GPSIMD only does small copies/adds (its big-op throughput is
    poor).
"""

import numpy as np

import concourse.bass as bass
import concourse.mybir as mybir
import concourse.tile as tile
from concourse.bass_utils import run_bass_kernel_spmd

B, T, E = 32, 2048, 128
NCORES = 8
BPC = B // NCORES            # batches per core
KT = T // 128                # token tiles per batch
CW = E + 2                   # row width: 128 x-cols + ones col + pad
F32 = mybir.dt.float32
F16 = mybir.dt.float16
AF = mybir.ActivationFunctionType
ALU = mybir.AluOpType
INV_T = 1.0 / T


def _patch_sem_clear():
    """The installed walrus cannot encode EVENT_SEMAPHORE_RANGE_CLEAR (raw
    ISA, "ISA wrong length"), which TileContext's exit path emits via
    gpsimd.sem_clear. Skip the clear (keep the DMA drain + bookkeeping);
    the runtime re-initializes semaphore state per NEFF execution."""
    if getattr(bass.Bass, "_semclear_patched", False):
        return
    from concourse.bass import compact_to_ranges

    def patched(self, sems):
        if not sems:
            return
        sem_nums = [s.num if hasattr(s, "num") else s for s in sems]
        for sem_range in compact_to_ranges(sem_nums):
            assert self._state.free_isdisjoint(sem_range)
            self.gpsimd.dma_reset(sem_range)
        self._state.prepend_free_semaphores(sem_nums)
        for poison_set in self._tile_sem_poison_stack:
            poison_set.update(sem_nums)

    bass.Bass.clear_and_free_semaphores = patched
    bass.Bass._semclear_patched = True


def _legalize_sync_waits(nc):
    """The installed walrus encodes at most one sync-wait per instruction
    ("Too many sync wait commands"). Move excess waits onto engine NoOps
    inserted immediately before the instruction — same engine, same
    program position, so semantics are unchanged."""
    import bass_rust

    fn = nc.m.functions[0]
    n_nops = 0
    for blk in fn.blocks:
        insts = blk.instructions
        out = []
        dirty = False
        for inst in insts:
            si = inst.sync_info
            if si is not None and len(si.on_wait) > 1:
                waits = list(si.on_wait)
                for w in waits[:-1]:
                    nop = mybir.InstNoOp(
                        name=f"waitnop-{n_nops}", engine=inst.engine
                    )
                    nop.sync_info = bass_rust.SyncInfo(
                        on_wait=[w], on_update=[]
                    )
                    out.append(nop)
                    n_nops += 1
                inst.sync_info = bass_rust.SyncInfo(
                    on_wait=[waits[-1]], on_update=list(si.on_update)
                )
                dirty = True
            out.append(inst)
        if dirty:
            blk.instructions = out
    return n_nops


def _build():
    _patch_sem_clear()
    nc = bass.Bass(
        "TRN2", target_bir_lowering=False, debug=False, num_devices=NCORES
    )

    # x: both sides packed, p-major tokens: row (b, s, p, k) = x_s[b, p*16+k]
    xd = nc.dram_tensor(
        "xc", (BPC * 2, 128, KT, CW), F16, kind="ExternalInput"
    ).ap()
    ud = nc.dram_tensor("u12", (2, E, T), F16, kind="ExternalInput").ap()
    wd = nc.dram_tensor("w12bc", (128, 2, E), F16, kind="ExternalInput").ap()
    bd = nc.dram_tensor("b12s", (128, 2, KT), F16, kind="ExternalInput").ap()
    outd = nc.dram_tensor("out", (2, BPC * E), F32, kind="ExternalOutput").ap()

    with tile.TileContext(nc) as tc:
        with (
            tc.tile_pool(name="const", bufs=1) as cpool,
            tc.tile_pool(name="xpool", bufs=1) as xpool,
            tc.tile_pool(name="work", bufs=2) as wpool,
            tc.tile_pool(name="ps", bufs=1, space="PSUM") as pspool,
        ):
            # ---- persistent tiles ----
            U12s = cpool.tile([128, 2, T], F16, tag="u12")
            W12 = cpool.tile([128, 2, E], F16, tag="w12")
            B12 = cpool.tile([128, 2, KT], F16, tag="b12")
            C12 = cpool.tile([128, 2, BPC], F16, tag="c12")
            XWB = cpool.tile([128, BPC, 2, KT], F16, tag="xwb")
            OUT = cpool.tile([128, BPC * E], F32, tag="outbuf")

            # small/param DMAs first on the ACT HWDGE ring
            nc.scalar.dma_start(W12[:], wd)
            nc.scalar.dma_start(B12[:], bd)

            # ---- x tiles: (128, 2, KT, CW); x1 on sync ring, x2 on scalar ----
            XB = []
            for b in range(BPC):
                xt = xpool.tile([128, 2, KT, CW], F16, tag=f"x_{b}")
                # batch 0 in quarters (earlier compute start), rest in halves
                nh = 4 if b == 0 else 2
                kq = KT // nh
                for h in range(nh):
                    ks = slice(h * kq, (h + 1) * kq)
                    nc.sync.dma_start(xt[:, 0, ks, :], xd[2 * b][:, ks])
                    nc.scalar.dma_start(xt[:, 1, ks, :], xd[2 * b + 1][:, ks])
                XB.append(xt)

            # U's load behind the x's (first needed only at the U-phase)
            nc.sync.dma_start(U12s[:, 0, :], ud[0])
            nc.scalar.dma_start(U12s[:, 1, :], ud[1])

            psE = pspool.tile([128, 2, KT, BPC], F32, tag="psE", bufs=1)

            # ---- per-batch: Gram phases, c's, x@W ----
            for b in range(BPC):
                xt = XB[b]

                # phase A: [G2 | t1] = x1^T @ [x2 | 1]
                psA = pspool.tile([128, CW], F32, tag="psA", bufs=2)
                for k in range(KT):
                    nc.tensor.matmul(
                        psA[:],
                        xt[:, 0, k, 0:E],
                        xt[:, 1, k, :],
                        start=(k == 0),
                        stop=(k == KT - 1),
                    )
                # phase B: [G | t2] = x2^T @ [x1 | 1]
                psB = pspool.tile([128, CW], F32, tag="psB", bufs=2)
                for k in range(KT):
                    nc.tensor.matmul(
                        psB[:],
                        xt[:, 1, k, 0:E],
                        xt[:, 0, k, :],
                        start=(k == 0),
                        stop=(k == KT - 1),
                    )

                GA = wpool.tile([128, CW], F16, tag="ga", bufs=2)
                GB = wpool.tile([128, CW], F16, tag="gb", bufs=2)
                nc.scalar.copy(GA[:], psA[:])
                nc.vector.tensor_copy(GB[:], psB[:])

                # TC = [t1 | t2]; then:
                #   lhsT=G2 (=GA), rhs=TC -> col1: G2^T t2 = T*c1
                #   lhsT=G  (=GB), rhs=TC -> col0: G^T t1  = T*c2
                TC = wpool.tile([128, 2], F16, tag="tc", bufs=2)
                nc.vector.tensor_copy(TC[:, 0:1], GA[:, E : E + 1])
                nc.vector.tensor_copy(TC[:, 1:2], GB[:, E : E + 1])
                psC = pspool.tile([128, 4], F32, tag="psC", bufs=1)
                nc.tensor.matmul(psC[:, 0:2], GA[:, 0:E], TC[:], start=True, stop=True)
                nc.tensor.matmul(psC[:, 2:4], GB[:, 0:E], TC[:], start=True, stop=True)
                # scale by 1/T while casting into the batched c matrix
                nc.vector.tensor_scalar_mul(C12[:, 0, b : b + 1], psC[:, 1:2], INV_T)
                nc.vector.tensor_scalar_mul(C12[:, 1, b : b + 1], psC[:, 2:3], INV_T)

                # x@W (+b): fused on DVE (fp16 fast modes), bias add on GPSIMD
                scr = wpool.tile([128, 2, KT, E], F16, tag="scr", bufs=2)
                xw = wpool.tile([128, 2, KT], F16, tag="xw", bufs=2)
                nc.vector.tensor_tensor(
                    scr[:],
                    xt[:, :, :, 0:E],
                    W12.unsqueeze(2).broadcast_to((128, 2, KT, E)),
                    ALU.mult,
                )
                with nc.allow_low_precision("fp16 xW reduce, |xW|~0.6"):
                    nc.vector.tensor_reduce(
                        out=xw[:], in_=scr[:], axis=mybir.AxisListType.X, op=ALU.add
                    )
                nc.gpsimd.tensor_add(XWB[:, b], xw[:], B12[:])

            # ---- U phase: et contributions for all batches at once ----
            # token t = p*16+k  ->  U column for (p, k) is U[:, p*16+k];
            # the host pre-permutes U so tile k's columns are contiguous.
            for s in range(2):
                for k in range(KT):
                    nc.tensor.matmul(
                        psE[:, s, k, :],
                        U12s[:, s, k * 128 : (k + 1) * 128],
                        C12[:, s, :],
                        start=True,
                        stop=True,
                    )

            # ---- per-batch: logits + per-partition maxima ----
            ETs = []
            mxall = cpool.tile([128, 2 * BPC], F32, tag="mxall")
            for b in range(BPC):
                et = wpool.tile([128, 2, KT], F32, tag="et", bufs=BPC)
                nc.vector.scalar_tensor_tensor(
                    out=et[:],
                    in0=psE[:, :, :, b],
                    scalar=1.0,
                    in1=XWB[:, b],
                    op0=ALU.mult,
                    op1=ALU.add,
                )
                # col j = 2*b + s
                nc.vector.tensor_reduce(
                    out=mxall[:, 2 * b : 2 * b + 2], in_=et[:],
                    axis=mybir.AxisListType.X, op=ALU.max,
                )
                ETs.append(et)

            # ---- cross-partition max (true per-side max subtraction keeps
            # exp() inside fp16 range for any data and makes softmax exact).
            # The installed walrus can't encode gpsimd partition reduces, so
            # fold on DVE: quadrant shuffles + 32x32 block transpose. ----
            idmask = list(range(32))
            macc = wpool.tile([32, 2 * BPC], F32, tag="macc")
            mtmp = wpool.tile([32, 2 * BPC], F32, tag="mtmp")
            nc.vector.tensor_copy(macc[:], mxall[0:32, :])
            for q in (1, 2, 3):
                nc.vector.stream_shuffle(
                    mtmp[:], mxall[32 * q : 32 * q + 32, :], mask=idmask
                )
                nc.vector.tensor_tensor(macc[:], macc[:], mtmp[:], ALU.max)
            p32 = wpool.tile([32, 32], F32, tag="mp32")
            nc.vector.memset(p32[:], -3.0e38)
            nc.vector.tensor_copy(p32[:, 0 : 2 * BPC], macc[:])
            t32 = wpool.tile([32, 32], F32, tag="mt32")
            nc.vector.transpose(t32[:], p32[:])
            nm = wpool.tile([32, 1], F32, tag="mnm")
            nc.vector.tensor_reduce(
                out=nm[:], in_=t32[:], axis=mybir.AxisListType.X, op=ALU.max
            )
            nmneg = wpool.tile([32, 1], F32, tag="mneg")
            nc.vector.tensor_scalar_mul(nmneg[:], nm[:], -1.0)
            # row-ify ([32,1] -> row 0 of [32,32]) then broadcast to all
            # partitions: shuffle mask 0 (every lane reads lane 0), then
            # quadrant copies.
            q32 = wpool.tile([32, 32], F32, tag="mq32")
            nc.vector.memset(q32[:], 0.0)
            nc.vector.tensor_copy(q32[:, 0:1], nmneg[:])
            r32 = wpool.tile([32, 32], F32, tag="mr32")
            nc.vector.transpose(r32[:], q32[:])
            nbias = cpool.tile([128, 32], F32, tag="nbias")
            nc.vector.stream_shuffle(nbias[0:32, :], r32[:], mask=[0] * 32)
            for q in (1, 2, 3):
                nc.vector.stream_shuffle(
                    nbias[32 * q : 32 * q + 32, :], nbias[0:32, :], mask=idmask
                )

            # ---- exp with per-(batch,side) bias ----
            EXs = []
            for b in range(BPC):
                EX = wpool.tile([128, 2, KT], F16, tag="ex", bufs=2)
                for s in range(2):
                    j = 2 * b + s
                    nc.scalar.activation(
                        EX[:, s, :], ETs[b][:, s, :], AF.Exp,
                        bias=nbias[:, j : j + 1],
                    )
                EXs.append(EX)

            # ---- readout in batch-pairs: 4 concurrent col-group streams ----
            # slot j = 2*s + bb (bb = b%2) -> PE col-group j, PSUM partition
            # 32*j.  Each slot accumulates EX_s^T [x_s | 1] over the 16
            # k-tiles; Z lands at col E via the ones column.
            for P in range(2):
                psO = pspool.tile([128, CW], F32, tag="psO", bufs=2)
                for k in range(KT):
                    for bb in range(2):
                        b = 2 * P + bb
                        for s in range(2):
                            j = 2 * s + bb
                            nc.tensor.matmul(
                                psO[32 * j : 32 * j + 1, :],
                                EXs[b][:, s, k : k + 1],
                                XB[b][:, s, k, :],
                                start=(k == 0),
                                stop=(k == KT - 1),
                                tile_position=(0, 32 * j),
                                skip_group_check=True,
                            )
                # normalize: out = o~ / Z
                rz = wpool.tile([128, 1], F32, tag="rz", bufs=2)
                for bb in range(2):
                    b = 2 * P + bb
                    for s in range(2):
                        j = 2 * s + bb
                        p0 = 32 * j
                        nc.vector.reciprocal(
                            rz[p0 : p0 + 1, :], psO[p0 : p0 + 1, E : E + 1]
                        )
                        nc.vector.tensor_scalar_mul(
                            OUT[p0 : p0 + 1, b * E : (b + 1) * E],
                            psO[p0 : p0 + 1, 0:E],
                            rz[p0 : p0 + 1, :],
                        )

            # out rows: side s batches (bb, bb+2) live on partition 32*(2s+bb)
            for s in range(2):
                for bb in range(2):
                    p0 = 32 * (2 * s + bb)
                    src = OUT[p0 : p0 + 1, :].rearrange(
                        "p (P bb e) -> p bb P e", bb=2, e=E
                    )
                    dst = outd[s].rearrange("(P bb e) -> bb P e", bb=2, e=E)
                    nc.sync.dma_start(
                        dst[bb].unsqueeze(0), src[:, bb]
                    )

    return nc


_NC_CACHE = {}


def _get_nc():
    if "nc" not in _NC_CACHE:
        _NC_CACHE["nc"] = _build()
    return _NC_CACHE["nc"]


# U column permutation: tile k, lane j  <-  U[:, j*16 + k]
_UIDX = np.arange(T).reshape(128, KT).T.reshape(-1)


def _prep_in_maps(x1, x2, W1, b1, U1, W2, b2, U2):
    f16 = np.float16
    x1 = np.asarray(x1, dtype=np.float32)
    x2 = np.asarray(x2, dtype=np.float32)

    # packed x: (B, 2, 128, KT, CW) fp16, token t = p*16 + k, ones col at E
    xall = np.zeros((B, 2, 128, KT, CW), dtype=f16)
    xall[:, 0, :, :, 0:E] = x1.reshape(B, 128, KT, E).astype(f16)
    xall[:, 1, :, :, 0:E] = x2.reshape(B, 128, KT, E).astype(f16)
    xall[:, :, :, :, E] = 1.0

    u12 = np.stack(
        [
            np.asarray(U1, np.float32)[:, _UIDX].astype(f16),
            np.asarray(U2, np.float32)[:, _UIDX].astype(f16),
        ]
    )
    w12 = np.ascontiguousarray(
        np.broadcast_to(
            np.stack(
                [np.asarray(W1, f16)[:, 0], np.asarray(W2, f16)[:, 0]]
            )[None, :, :],
            (128, 2, E),
        )
    )
    b12 = np.ascontiguousarray(
        np.stack(
            [
                np.asarray(b1, f16)[:, 0].reshape(128, KT),
                np.asarray(b2, f16)[:, 0].reshape(128, KT),
            ],
            axis=1,
        )
    )

    in_maps = []
    for c in range(NCORES):
        sl = slice(c * BPC, (c + 1) * BPC)
        in_maps.append(
            {
                "xc": np.ascontiguousarray(xall[sl]).reshape(
                    BPC * 2, 128, KT, CW
                ),
                "u12": u12,
                "w12bc": w12,
                "b12s": b12,
            }
        )
    return in_maps


def _run(trace=False, tmpdir=None, **inputs):
    nc = _get_nc()
    if not _NC_CACHE.get("legalized"):
        # must happen after any CoreSim use (sim can't model bare wait-nops)
        _legalize_sync_waits(nc)
        _NC_CACHE["legalized"] = True
    in_maps = _prep_in_maps(**inputs)
    res = run_bass_kernel_spmd(
        nc, in_maps, list(range(NCORES)), trace=trace, tmpdir=tmpdir
    )
    # per-core out: (2, BPC*E) -> (BPC, 2E)
    outs = []
    for r in res.results:
        o = r["out"].reshape(2, BPC, E)
        outs.append(np.concatenate([o[0], o[1]], axis=1))
    out = np.concatenate(outs, axis=0)
    return out, res


def kernel(x1, x2, W1, b1, U1, W2, b2, U2):
    out, _ = _run(
        x1=x1, x2=x2, W1=W1, b1=b1, U1=U1, W2=W2, b2=b2, U2=U2
    )
    return out


# revision 15
# speedup vs baseline: 1.6725x; 1.0904x over previous
"""AttentionMM kernel for Trainium2 (Bass/Tile), data-parallel over 8 NeuronCores.

Math (per batch b, with x1,x2: (T,E)):
    S = x1 @ x2^T  is never materialized:
        t1 = sum_i x1[i,:] ;  t2 = sum_j x2[j,:]
        G2 = x1^T @ x2  (E,E)
        c1 = (1/T) G2^T t2 ;  c2 = (1/T) G2 t1   (via G = x2^T x1 = G2^T)
    et1 = c1 @ U1 + x1 @ W1 + b1 ;  et2 = c2 @ U2 + x2 @ W2 + b2
    o1 = softmax(et1) @ x1 ;  o2 = softmax(et2) @ x2 ;  out = [o1 | o2]

Implementation notes:
  - Everything on-chip is float16: fp16 matmuls run at 1 cycle/row on the PE
    (vs 4 cycles/row for f32r below 256 moving columns) and halve HBM
    traffic.  PSUM accumulation stays fp32, so the only precision loss is
    input rounding (2^-11) - measured end-to-end rel err ~7e-3.
  - Tokens sit in SBUF partitions, p-major: token t = p*16 + k, so each
    DMA moves 16 consecutive 260B rows per partition (4KB+ contiguous
    chunks on both sides -> good HBM descriptor efficiency).
  - The host appends a ones-column to x (E -> 129 cols, padded to 130):
    Gram matmuls then yield the token-sums t1/t2 for free, and readout
    matmuls yield the softmax denominator Z for free.
  - Softmax uses a constant shift (-27) instead of a max-subtraction.
    The shift cancels exactly in o = (sum ex*x)/Z; -27 keeps every
    batch's exp() inside fp16 range (global max logit ~37 -> e^10~22K
    < 65504; weakest batch max logit ~15 -> e^-11.6 ~ 9e-6, still ~150
    denormal steps).
  - Both attention sides are packed side by side ([x1|1|0|x2|1|0] per
    token) so per-batch elementwise work is one DVE op per stage and the
    readout runs as a single M=2, N=260 accumulating matmul per k-tile.
  - x@W runs on DVE (fused multiply then X-axis reduce, fp16 2x/4x
    modes); GPSIMD only does small copies/adds (its big-op throughput is
    poor).
"""

import numpy as np

import concourse.bass as bass
import concourse.mybir as mybir
import concourse.tile as tile
from concourse.bass_utils import run_bass_kernel_spmd

B, T, E = 32, 2048, 128
NCORES = 8
BPC = B // NCORES            # batches per core
KT = T // 128                # token tiles per batch
CW = E + 2                   # row width: 128 x-cols + ones col + pad
F32 = mybir.dt.float32
F16 = mybir.dt.float16
AF = mybir.ActivationFunctionType
ALU = mybir.AluOpType
INV_T = 1.0 / T


def _patch_sem_clear():
    """The installed walrus cannot encode EVENT_SEMAPHORE_RANGE_CLEAR (raw
    ISA, "ISA wrong length"), which TileContext's exit path emits via
    gpsimd.sem_clear. Skip the clear (keep the DMA drain + bookkeeping);
    the runtime re-initializes semaphore state per NEFF execution."""
    if getattr(bass.Bass, "_semclear_patched", False):
        return
    from concourse.bass import compact_to_ranges

    def patched(self, sems):
        if not sems:
            return
        sem_nums = [s.num if hasattr(s, "num") else s for s in sems]
        for sem_range in compact_to_ranges(sem_nums):
            assert self._state.free_isdisjoint(sem_range)
            self.gpsimd.dma_reset(sem_range)
        self._state.prepend_free_semaphores(sem_nums)
        for poison_set in self._tile_sem_poison_stack:
            poison_set.update(sem_nums)

    bass.Bass.clear_and_free_semaphores = patched
    bass.Bass._semclear_patched = True


def _legalize_sync_waits(nc):
    """The installed walrus encodes at most one sync-wait per instruction
    ("Too many sync wait commands"). Move excess waits onto engine NoOps
    inserted immediately before the instruction — same engine, same
    program position, so semantics are unchanged."""
    import bass_rust

    fn = nc.m.functions[0]
    n_nops = 0
    for blk in fn.blocks:
        insts = blk.instructions
        out = []
        dirty = False
        for inst in insts:
            si = inst.sync_info
            if si is not None and len(si.on_wait) > 1:
                waits = list(si.on_wait)
                for w in waits[:-1]:
                    nop = mybir.InstNoOp(
                        name=f"waitnop-{n_nops}", engine=inst.engine
                    )
                    nop.sync_info = bass_rust.SyncInfo(
                        on_wait=[w], on_update=[]
                    )
                    out.append(nop)
                    n_nops += 1
                inst.sync_info = bass_rust.SyncInfo(
                    on_wait=[waits[-1]], on_update=list(si.on_update)
                )
                dirty = True
            out.append(inst)
        if dirty:
            blk.instructions = out
    return n_nops


def _build():
    _patch_sem_clear()
    nc = bass.Bass(
        "TRN2", target_bir_lowering=False, debug=False, num_devices=NCORES
    )

    # x: both sides packed, p-major tokens: row (b, s, p, k) = x_s[b, p*16+k]
    xd = nc.dram_tensor(
        "xc", (BPC * 2, 128, KT, CW), F16, kind="ExternalInput"
    ).ap()
    ud = nc.dram_tensor("u12", (2, E, T), F16, kind="ExternalInput").ap()
    wd = nc.dram_tensor("w12bc", (128, 2, E), F16, kind="ExternalInput").ap()
    bd = nc.dram_tensor("b12s", (128, 2, KT), F16, kind="ExternalInput").ap()
    outd = nc.dram_tensor("out", (2, BPC * E), F32, kind="ExternalOutput").ap()

    with tile.TileContext(nc) as tc:
        with (
            tc.tile_pool(name="const", bufs=1) as cpool,
            tc.tile_pool(name="xpool", bufs=1) as xpool,
            tc.tile_pool(name="work", bufs=2) as wpool,
            tc.tile_pool(name="ps", bufs=1, space="PSUM") as pspool,
        ):
            # ---- persistent tiles ----
            U12s = cpool.tile([128, 2, T], F16, tag="u12")
            W12 = cpool.tile([128, 2, E], F16, tag="w12")
            B12 = cpool.tile([128, 2, KT], F16, tag="b12")
            C12 = cpool.tile([128, 2, BPC], F16, tag="c12")
            XWB = cpool.tile([128, BPC, 2, KT], F16, tag="xwb")
            OUT = cpool.tile([128, BPC * E], F32, tag="outbuf")

            # small/param DMAs first on the sync HWDGE ring
            nc.sync.dma_start(W12[:], wd)
            nc.sync.dma_start(B12[:], bd)

            # ---- x tiles: (128, 2, KT, CW) ----
            # x1 on the sync ring; x2 for batch 0 on the scalar ring (needed
            # first; ACT must stay free for compute afterwards), x2 for
            # batches 1-3 on the GPSIMD SWDGE ring.
            XB = []
            for b in range(BPC):
                xt = xpool.tile([128, 2, KT, CW], F16, tag=f"x_{b}")
                # batch 0 in quarters (earlier compute start), rest in halves
                nh = 4 if b == 0 else 2
                kq = KT // nh
                for h in range(nh):
                    ks = slice(h * kq, (h + 1) * kq)
                    nc.sync.dma_start(xt[:, 0, ks, :], xd[2 * b][:, ks])
                    eng = nc.scalar if b == 0 else nc.gpsimd
                    eng.dma_start(xt[:, 1, ks, :], xd[2 * b + 1][:, ks])
                XB.append(xt)

                # x@W: issued with the loads so DVE starts early.  Multiply
                # at fp16 2x rate, then a tensor_tensor fold tree (DVE's
                # plain reduce runs at 1x; the tree stays at 2x throughout).
                scr = wpool.tile([128, 2, KT, E], F16, tag="scr", bufs=2)
                nc.vector.tensor_tensor(
                    scr[:],
                    xt[:, :, :, 0:E],
                    W12.unsqueeze(2).broadcast_to((128, 2, KT, E)),
                    ALU.mult,
                )
                src = scr
                w = E
                while w > 1:
                    h = w // 2
                    dst = wpool.tile([128, 2, KT, h], F16, tag=f"xf{h}", bufs=2)
                    nc.vector.tensor_tensor(
                        dst[:], src[:, :, :, 0:h], src[:, :, :, h:w], ALU.add
                    )
                    src = dst
                    w = h
                nc.gpsimd.tensor_add(XWB[:, b], src[:, :, :, 0], B12[:])

            # U's load behind the x's (first needed only at the U-phase)
            nc.sync.dma_start(U12s[:, 0, :], ud[0])
            nc.sync.dma_start(U12s[:, 1, :], ud[1])

            psE = pspool.tile([128, 2, KT, BPC], F32, tag="psE", bufs=1)

            # ---- per-batch: Gram phases, c's ----
            for b in range(BPC):
                xt = XB[b]

                # phase A: [G2 | t1] = x1^T @ [x2 | 1]
                psA = pspool.tile([128, CW], F32, tag="psA", bufs=2)
                for k in range(KT):
                    nc.tensor.matmul(
                        psA[:],
                        xt[:, 0, k, 0:E],
                        xt[:, 1, k, :],
                        start=(k == 0),
                        stop=(k == KT - 1),
                    )
                # phase B: [G | t2] = x2^T @ [x1 | 1]
                psB = pspool.tile([128, CW], F32, tag="psB", bufs=2)
                for k in range(KT):
                    nc.tensor.matmul(
                        psB[:],
                        xt[:, 1, k, 0:E],
                        xt[:, 0, k, :],
                        start=(k == 0),
                        stop=(k == KT - 1),
                    )

                GA = wpool.tile([128, CW], F16, tag="ga", bufs=2)
                GB = wpool.tile([128, CW], F16, tag="gb", bufs=2)
                nc.scalar.copy(GA[:], psA[:])
                nc.scalar.copy(GB[:], psB[:])

                # TC = [t1 | t2]; then:
                #   lhsT=G2 (=GA), rhs=TC -> col1: G2^T t2 = T*c1
                #   lhsT=G  (=GB), rhs=TC -> col0: G^T t1  = T*c2
                TC = wpool.tile([128, 2], F16, tag="tc", bufs=2)
                nc.gpsimd.tensor_copy(TC[:, 0:1], GA[:, E : E + 1])
                nc.gpsimd.tensor_copy(TC[:, 1:2], GB[:, E : E + 1])
                psC = pspool.tile([128, 4], F32, tag="psC", bufs=1)
                nc.tensor.matmul(psC[:, 0:2], GA[:, 0:E], TC[:], start=True, stop=True)
                nc.tensor.matmul(psC[:, 2:4], GB[:, 0:E], TC[:], start=True, stop=True)
                # scale by 1/T while casting into the batched c matrix
                nc.scalar.mul(C12[:, 0, b : b + 1], psC[:, 1:2], INV_T)
                nc.scalar.mul(C12[:, 1, b : b + 1], psC[:, 2:3], INV_T)

            # ---- U phase: et contributions for all batches at once ----
            # token t = p*16+k  ->  U column for (p, k) is U[:, p*16+k];
            # the host pre-permutes U so tile k's columns are contiguous.
            for s in range(2):
                for k in range(KT):
                    nc.tensor.matmul(
                        psE[:, s, k, :],
                        U12s[:, s, k * 128 : (k + 1) * 128],
                        C12[:, s, :],
                        start=True,
                        stop=True,
                    )

            # ---- per-batch: logits + per-partition maxima ----
            ETs = []
            mxall = cpool.tile([128, 2 * BPC], F32, tag="mxall")
            for b in range(BPC):
                et = wpool.tile([128, 2, KT], F32, tag="et", bufs=BPC)
                nc.vector.scalar_tensor_tensor(
                    out=et[:],
                    in0=psE[:, :, :, b],
                    scalar=1.0,
                    in1=XWB[:, b],
                    op0=ALU.mult,
                    op1=ALU.add,
                )
                # col j = 2*b + s
                nc.vector.tensor_reduce(
                    out=mxall[:, 2 * b : 2 * b + 2], in_=et[:],
                    axis=mybir.AxisListType.X, op=ALU.max,
                )
                ETs.append(et)

            # ---- cross-partition max (true per-side max subtraction keeps
            # exp() inside fp16 range for any data and makes softmax exact).
            # The installed walrus can't encode gpsimd partition reduces, so
            # fold on DVE: quadrant shuffles + 32x32 block transpose. ----
            idmask = list(range(32))
            macc = wpool.tile([32, 2 * BPC], F32, tag="macc")
            mtmp = wpool.tile([32, 2 * BPC], F32, tag="mtmp")
            nc.vector.tensor_copy(macc[:], mxall[0:32, :])
            for q in (1, 2, 3):
                nc.vector.stream_shuffle(
                    mtmp[:], mxall[32 * q : 32 * q + 32, :], mask=idmask
                )
                nc.vector.tensor_tensor(macc[:], macc[:], mtmp[:], ALU.max)
            p32 = wpool.tile([32, 32], F32, tag="mp32")
            nc.vector.memset(p32[:], -3.0e38)
            nc.vector.tensor_copy(p32[:, 0 : 2 * BPC], macc[:])
            t32 = wpool.tile([32, 32], F32, tag="mt32")
            nc.vector.transpose(t32[:], p32[:])
            nm = wpool.tile([32, 1], F32, tag="mnm")
            nc.vector.tensor_reduce(
                out=nm[:], in_=t32[:], axis=mybir.AxisListType.X, op=ALU.max
            )
            nmneg = wpool.tile([32, 1], F32, tag="mneg")
            nc.vector.tensor_scalar_mul(nmneg[:], nm[:], -1.0)
            # row-ify ([32,1] -> row 0 of [32,32]) then broadcast to all
            # partitions: shuffle mask 0 (every lane reads lane 0), then
            # quadrant copies.
            q32 = wpool.tile([32, 32], F32, tag="mq32")
            nc.vector.memset(q32[:], 0.0)
            nc.vector.tensor_copy(q32[:, 0:1], nmneg[:])
            r32 = wpool.tile([32, 32], F32, tag="mr32")
            nc.vector.transpose(r32[:], q32[:])
            nbias = cpool.tile([128, 32], F32, tag="nbias")
            nc.vector.stream_shuffle(nbias[0:32, :], r32[:], mask=[0] * 32)
            for q in (1, 2, 3):
                nc.vector.stream_shuffle(
                    nbias[32 * q : 32 * q + 32, :], nbias[0:32, :], mask=idmask
                )

            # ---- exp with per-(batch,side) bias ----
            EXs = []
            for b in range(BPC):
                EX = wpool.tile([128, 2, KT], F16, tag="ex", bufs=2)
                for s in range(2):
                    j = 2 * b + s
                    nc.scalar.activation(
                        EX[:, s, :], ETs[b][:, s, :], AF.Exp,
                        bias=nbias[:, j : j + 1],
                    )
                EXs.append(EX)

            # ---- readout in batch-pairs: 4 concurrent col-group streams ----
            # slot j = 2*s + bb (bb = b%2) -> PE col-group j, PSUM partition
            # 32*j.  Each slot accumulates EX_s^T [x_s | 1] over the 16
            # k-tiles; Z lands at col E via the ones column.
            for P in range(2):
                psO = pspool.tile([128, CW], F32, tag="psO", bufs=2)
                for k in range(KT):
                    for bb in range(2):
                        b = 2 * P + bb
                        for s in range(2):
                            j = 2 * s + bb
                            nc.tensor.matmul(
                                psO[32 * j : 32 * j + 1, :],
                                EXs[b][:, s, k : k + 1],
                                XB[b][:, s, k, :],
                                start=(k == 0),
                                stop=(k == KT - 1),
                                tile_position=(0, 32 * j),
                                skip_group_check=True,
                            )
                # normalize: out = o~ / Z
                rz = wpool.tile([128, 1], F32, tag="rz", bufs=2)
                for bb in range(2):
                    b = 2 * P + bb
                    for s in range(2):
                        j = 2 * s + bb
                        p0 = 32 * j
                        nc.vector.reciprocal(
                            rz[p0 : p0 + 1, :], psO[p0 : p0 + 1, E : E + 1]
                        )
                        nc.scalar.mul(
                            OUT[p0 : p0 + 1, b * E : (b + 1) * E],
                            psO[p0 : p0 + 1, 0:E],
                            rz[p0 : p0 + 1, :],
                        )

            # out rows: side s batches (bb, bb+2) live on partition 32*(2s+bb)
            for s in range(2):
                for bb in range(2):
                    p0 = 32 * (2 * s + bb)
                    src = OUT[p0 : p0 + 1, :].rearrange(
                        "p (P bb e) -> p bb P e", bb=2, e=E
                    )
                    dst = outd[s].rearrange("(P bb e) -> bb P e", bb=2, e=E)
                    nc.sync.dma_start(
                        dst[bb].unsqueeze(0), src[:, bb]
                    )

    return nc


_NC_CACHE = {}


def _get_nc():
    if "nc" not in _NC_CACHE:
        _NC_CACHE["nc"] = _build()
    return _NC_CACHE["nc"]


# U column permutation: tile k, lane j  <-  U[:, j*16 + k]
_UIDX = np.arange(T).reshape(128, KT).T.reshape(-1)


def _prep_in_maps(x1, x2, W1, b1, U1, W2, b2, U2):
    f16 = np.float16
    x1 = np.asarray(x1, dtype=np.float32)
    x2 = np.asarray(x2, dtype=np.float32)

    # packed x: (B, 2, 128, KT, CW) fp16, token t = p*16 + k, ones col at E
    xall = np.zeros((B, 2, 128, KT, CW), dtype=f16)
    xall[:, 0, :, :, 0:E] = x1.reshape(B, 128, KT, E).astype(f16)
    xall[:, 1, :, :, 0:E] = x2.reshape(B, 128, KT, E).astype(f16)
    xall[:, :, :, :, E] = 1.0

    u12 = np.stack(
        [
            np.asarray(U1, np.float32)[:, _UIDX].astype(f16),
            np.asarray(U2, np.float32)[:, _UIDX].astype(f16),
        ]
    )
    w12 = np.ascontiguousarray(
        np.broadcast_to(
            np.stack(
                [np.asarray(W1, f16)[:, 0], np.asarray(W2, f16)[:, 0]]
            )[None, :, :],
            (128, 2, E),
        )
    )
    b12 = np.ascontiguousarray(
        np.stack(
            [
                np.asarray(b1, f16)[:, 0].reshape(128, KT),
                np.asarray(b2, f16)[:, 0].reshape(128, KT),
            ],
            axis=1,
        )
    )

    in_maps = []
    for c in range(NCORES):
        sl = slice(c * BPC, (c + 1) * BPC)
        in_maps.append(
            {
                "xc": np.ascontiguousarray(xall[sl]).reshape(
                    BPC * 2, 128, KT, CW
                ),
                "u12": u12,
                "w12bc": w12,
                "b12s": b12,
            }
        )
    return in_maps


def _run(trace=False, tmpdir=None, **inputs):
    nc = _get_nc()
    if not _NC_CACHE.get("legalized"):
        # must happen after any CoreSim use (sim can't model bare wait-nops)
        _legalize_sync_waits(nc)
        _NC_CACHE["legalized"] = True
    in_maps = _prep_in_maps(**inputs)
    res = run_bass_kernel_spmd(
        nc, in_maps, list(range(NCORES)), trace=trace, tmpdir=tmpdir
    )
    # per-core out: (2, BPC*E) -> (BPC, 2E)
    outs = []
    for r in res.results:
        o = r["out"].reshape(2, BPC, E)
        outs.append(np.concatenate([o[0], o[1]], axis=1))
    out = np.concatenate(outs, axis=0)
    return out, res


def kernel(x1, x2, W1, b1, U1, W2, b2, U2):
    out, _ = _run(
        x1=x1, x2=x2, W1=W1, b1=b1, U1=U1, W2=W2, b2=b2, U2=U2
    )
    return out
